# revision 5
# baseline (speedup 1.0000x reference)
"""Trainium2 Bass kernel for nn_MoEsparseRoutingForClassification.

Reference computation (B=64, S=128, H=1024, E=8, L=2):
    x = X[:, 0, :]                                   # CLS token [B,H]
    y[b,o]   = sum_e g[b,e] * (x[b] . dense_w[e,o,:]) + (g @ dense_b)[b,o]
    t        = tanh(y)
    out[b,l] = sum_e g[b,e] * (t[b] . out_w[e,l,:])  + (g @ out_b)[b,l]

Distribution: the H output dim of the dense layer is sharded 8 ways
(OC=128 per core).  Core c computes y[:, c*OC:(c+1)*OC] (which needs the
full CLS token but only a 4 MiB slice dense_w[:, c_slice, :]), applies
tanh, and contracts its slice against out_w[:, :, c_slice] to produce a
partial [B,L] logit.  The partials (incl. the out_b bias, fed only to
core 0) sum to the full output on the host.  Total HBM traffic per core
is ~4.3 MiB - the memory roofline for this problem - and no cross-core
collective is needed.

Everything arithmetic runs on device; the host only slices, transposes
(layout prep), and sums the 8 partial [64,2] outputs.
"""

import sys

import numpy as np

for _p in ("/opt/trn_rl_repo",):
    if _p not in sys.path:
        sys.path.insert(0, _p)

# If the environment sets BASS_TRACE but lacks antenv.axon_hooks (this agent
# image does), run_bass_kernel_spmd would crash on import; pre-seed a no-op
# module so tracing degrades gracefully instead.
try:  # pragma: no cover
    import antenv.axon_hooks  # noqa: F401
except Exception:  # pragma: no cover
    import types as _types

    _m = _types.ModuleType("antenv.axon_hooks")
    _m._hook = None
    _m.set_axon_ntff_profile_hook = lambda h: setattr(_m, "_hook", h)
    _m.get_axon_ntff_profile_hook = lambda: _m._hook
    sys.modules["antenv.axon_hooks"] = _m

B, S, H = 64, 128, 1024
E, L = 8, 2
NCORES = 8
OC = H // NCORES          # dense-output slice per core
KT = H // 128             # contraction tiles
P = 128

# Stage-1 matmul layout:
#   "A": psum [B, (e,oc)]  - lhsT = xT k-tile  [128,  64], 16 MMs of N=512
#   "B": psum [(oc), (e,b)] - lhsT = w1 block  [128, 128], 64 MMs of N=64
LAYOUT = "A"

_cached = None


def _build(layout=LAYOUT):
    from contextlib import ExitStack

    import concourse.bass as bass
    import concourse.tile as tile
    from concourse import bacc, mybir

    F32 = mybir.dt.float32
    AF = mybir.ActivationFunctionType
    OP = mybir.AluOpType

    nc = bacc.Bacc("TRN2", target_bir_lowering=False, debug=False,
                   num_devices=NCORES)

    xt_d = nc.dram_tensor("xt", [P, KT, B], F32, kind="ExternalInput")
    w1_d = nc.dram_tensor("w1", [P, KT, E, OC], F32, kind="ExternalInput")
    gt_d = nc.dram_tensor("gt", [E, B], F32, kind="ExternalInput")
    gc_d = nc.dram_tensor("gc", [B, E], F32, kind="ExternalInput")
    db_d = nc.dram_tensor("db", [E, OC], F32, kind="ExternalInput")
    ow_d = nc.dram_tensor("ow", [E, L * OC], F32, kind="ExternalInput")
    ob_d = nc.dram_tensor("ob", [E, L], F32, kind="ExternalInput")
    out_d = nc.dram_tensor("out", [B, L], F32, kind="ExternalOutput")

    with tile.TileContext(nc) as tc, ExitStack() as ctx:
        consts = ctx.enter_context(tc.tile_pool(name="consts", bufs=1))
        wpool = ctx.enter_context(tc.tile_pool(name="wpool", bufs=1))
        mixp = ctx.enter_context(tc.tile_pool(name="mixp", bufs=2))
        smallp = ctx.enter_context(tc.tile_pool(name="smallp", bufs=1))
        psy = ctx.enter_context(tc.tile_pool(name="psy", bufs=1, space="PSUM"))
        pss = ctx.enter_context(tc.tile_pool(name="pss", bufs=1, space="PSUM"))

        # small constants via SWDGE (gpsimd) so they don't queue behind w1
        gt_t = consts.tile([E, B], F32)
        nc.gpsimd.dma_start(out=gt_t, in_=gt_d.ap())
        gc_t = consts.tile([B, E], F32)
        nc.gpsimd.dma_start(out=gc_t, in_=gc_d.ap())
        db_t = consts.tile([E, OC], F32)
        nc.gpsimd.dma_start(out=db_t, in_=db_d.ap())
        ow_t = consts.tile([E, L * OC], F32)
        nc.gpsimd.dma_start(out=ow_t, in_=ow_d.ap())
        ob_t = consts.tile([E, L], F32)
        nc.gpsimd.dma_start(out=ob_t, in_=ob_d.ap())

        xt_t = consts.tile([P, KT, B], F32)
        nc.sync.dma_start(out=xt_t, in_=xt_d.ap())

        # big weight slice, chunked by k so matmuls can start early
        w1_t = wpool.tile([P, KT, E, OC], F32)
        for k in range(KT):
            nc.sync.dma_start(out=w1_t[:, k], in_=w1_d.ap()[:, k])

        # ---- stage 1: y_e[b, oc] = x . dense_w[e, oc_slice, :] ----
        if layout == "A":
            psum_y = psy.tile([B, E, OC], F32)          # [64, 8, 128]
            for k in range(KT):
                for h in range(2):
                    nc.tensor.matmul(
                        psum_y[:, h * 4:(h + 1) * 4, :],
                        xt_t[:, k, :],
                        w1_t[:, k, h * 4:(h + 1) * 4, :],
                        start=(k == 0),
                        stop=(k == KT - 1),
                    )
        else:
            psum_y = psy.tile([P, E, B], F32)           # [128, 8, 64] (y^T)
            for k in range(KT):
                for e in range(E):
                    nc.tensor.matmul(
                        psum_y[:, e, :],
                        w1_t[:, k, e, :],
                        xt_t[:, k, :],
                        start=(k == 0),
                        stop=(k == KT - 1),
                    )

        # ---- dense bias: sel_db[b, oc] = (g @ dense_b)[b, oc_slice] ----
        if layout == "A":
            psum_db = pss.tile([B, OC], F32)
            nc.tensor.matmul(psum_db[:], gt_t[:], db_t[:], start=True, stop=True)
        else:
            psum_db = pss.tile([OC, B], F32)            # sel_db^T
            nc.tensor.matmul(psum_db[:], db_t[:], gt_t[:], start=True, stop=True)

        # ---- gate mix + tanh ----
        if layout == "A":
            # acc = sel_db + sum_e g[:,e] * y_e   (per-partition scalar FMA)
            # stt can read at most one non-scalar PSUM input, so sel_db goes
            # through SBUF first (ACT copy, off the critical path).
            sdb_t = smallp.tile([B, OC], F32)
            nc.scalar.copy(sdb_t[:], psum_db[:])
            acc = sdb_t
            for e in range(E):
                nxt = mixp.tile([B, OC], F32, tag="acc")
                nc.vector.scalar_tensor_tensor(
                    out=nxt,
                    in0=psum_y[:, e, :],
                    scalar=gc_t[:, e:e + 1],
                    in1=acc[:],
                    op0=OP.mult,
                    op1=OP.add,
                )
                acc = nxt
            t_t = smallp.tile([B, OC], F32)
            nc.scalar.activation(t_t[:], acc[:], AF.Tanh)
        else:
            # y^T mix: need gates broadcast over partitions: gb[p,(e,b)]=g[b,e]
            gb_t = consts.tile([P, E * B], F32)
            gb_src = bass.AP(
                tensor=gt_d.ap().tensor, offset=0,
                ap=[[0, P], [1, E * B]],
            )
            nc.gpsimd.dma_start(out=gb_t, in_=gb_src)
            prod = mixp.tile([P, E, B], F32)
            nc.vector.tensor_tensor(
                out=prod[:], in0=psum_y[:], in1=gb_t[:].rearrange(
                    "p (e b) -> p e b", e=E),
                op=OP.mult,
            )
            ytT = mixp.tile([P, B], F32)
            nc.vector.tensor_reduce(
                out=ytT,
                in_=prod[:].rearrange("p e b -> p b e"),
                axis=mybir.AxisListType.X,
                op=OP.add,
            )
            y2 = mixp.tile([P, B], F32)
            nc.vector.tensor_add(y2[:], ytT[:], psum_db[:])
            tT_t = smallp.tile([P, B], F32)
            nc.scalar.activation(tT_t[:], y2[:], AF.Tanh)
            # transpose t^T [oc, b] -> t [b, oc] via PE
            from concourse.masks import make_identity
            ident = consts.tile([P, P], F32)
            make_identity(nc, ident)
            psum_tr = pss.tile([B, P], F32)
            nc.tensor.transpose(psum_tr[:], tT_t[:], ident[:])
            t_t = smallp.tile([B, OC], F32)
            nc.vector.tensor_copy(t_t[:], psum_tr[:])

        # ---- stage 2 ----
        # sel_ow[b, (l, oc)] = sum_e g[b,e] out_w[e, l, oc_slice]
        psum_ow = pss.tile([B, L, OC], F32)
        nc.tensor.matmul(
            psum_ow[:].rearrange("b l o -> b (l o)"),
            gt_t[:], ow_t[:], start=True, stop=True,
        )
        psum_ob = pss.tile([B, L], F32)
        nc.tensor.matmul(psum_ob[:], gt_t[:], ob_t[:], start=True, stop=True)
        sob_t = smallp.tile([B, L], F32)
        nc.scalar.copy(sob_t[:], psum_ob[:])

        # partial[b,l] = sum_oc t[b,oc] * sel_ow[b,l,oc]  (+ sel_ob)
        # NOTE: InstTensorTensorReduce faults TRN2 here; scalar_tensor_tensor
        # with accum_out (free-dim sum) is the reliable path.
        out_t = smallp.tile([B, L], F32)
        pre_t = smallp.tile([B, L], F32)
        dump = smallp.tile([B, OC], F32)
        for l in range(L):
            nc.vector.scalar_tensor_tensor(
                out=dump[:],
                in0=psum_ow[:, l, :],
                scalar=1.0,
                in1=t_t[:],
                op0=OP.mult,
                op1=OP.mult,
                accum_out=pre_t[:, l:l + 1],
            )
        nc.vector.tensor_add(out_t[:], pre_t[:], sob_t[:])

        nc.sync.dma_start(out=out_d.ap(), in_=out_t[:])

    nc.compile()
    return nc


def _prep_inputs(X, gates, dense_w, dense_b, out_w, out_b):
    """Host-side layout prep (slice/transpose only) -> per-core input maps."""
    X = np.asarray(X, dtype=np.float32)
    gates = np.asarray(gates, dtype=np.float32)
    dense_w = np.asarray(dense_w, dtype=np.float32)
    dense_b = np.asarray(dense_b, dtype=np.float32)
    out_w = np.asarray(out_w, dtype=np.float32)
    out_b = np.asarray(out_b, dtype=np.float32)

    xcls = X[:, 0, :]                                     # [B, H]
    # xt[i_lo, k, b] = x[b, k*128 + i_lo]
    xt = np.ascontiguousarray(xcls.T.reshape(KT, P, B).transpose(1, 0, 2))
    gt = np.ascontiguousarray(gates.T)                    # [E, B]
    gc = np.ascontiguousarray(gates)                      # [B, E]

    in_maps = []
    for c in range(NCORES):
        sl = slice(c * OC, (c + 1) * OC)
        # w1[i_lo, k, e, oc] = dense_w[e, c*OC + oc, k*128 + i_lo]
        w1 = np.ascontiguousarray(
            dense_w[:, sl, :].reshape(E, OC, KT, P).transpose(3, 2, 0, 1)
        )
        ow = np.ascontiguousarray(out_w[:, :, sl].reshape(E, L * OC))
        ob = out_b if c == 0 else np.zeros_like(out_b)
        in_maps.append({
            "xt": xt,
            "w1": w1,
            "gt": gt,
            "gc": gc,
            "db": np.ascontiguousarray(dense_b[:, sl]),
            "ow": ow,
            "ob": np.ascontiguousarray(ob),
        })
    return in_maps


def _run(in_maps, trace=False, tmpdir=None):
    global _cached
    from concourse.bass_utils import run_bass_kernel_spmd

    if _cached is None:
        _cached = _build()
    res = run_bass_kernel_spmd(
        _cached, in_maps, list(range(NCORES)), trace=trace, tmpdir=tmpdir,
    )
    return res


def kernel(X, gates, dense_w, dense_b, out_w, out_b):
    in_maps = _prep_inputs(X, gates, dense_w, dense_b, out_w, out_b)
    res = _run(in_maps)
    acc = np.zeros((B, L), dtype=np.float64)
    for c in range(NCORES):
        acc += res.results[c]["out"].astype(np.float64)
    return acc.astype(np.float32)


# revision 6
# speedup vs baseline: 1.0153x; 1.0153x over previous
"""Trainium2 Bass kernel for nn_MoEsparseRoutingForClassification.

Reference computation (B=64, S=128, H=1024, E=8, L=2):
    x = X[:, 0, :]                                   # CLS token [B,H]
    y[b,o]   = sum_e g[b,e] * (x[b] . dense_w[e,o,:]) + (g @ dense_b)[b,o]
    t        = tanh(y)
    out[b,l] = sum_e g[b,e] * (t[b] . out_w[e,l,:])  + (g @ out_b)[b,l]

Distribution: the H output dim of the dense layer is sharded 8 ways
(OC=128 per core).  Core c computes y[:, c*OC:(c+1)*OC] (which needs the
full CLS token but only a 4 MiB slice dense_w[:, c_slice, :]), applies
tanh, and contracts its slice against out_w[:, :, c_slice] to produce a
partial [B,L] logit.  The partials (incl. the out_b bias, fed only to
core 0) sum to the full output on the host.  Total HBM traffic per core
is ~4.3 MiB - the memory roofline for this problem - and no cross-core
collective is needed.

Everything arithmetic runs on device; the host only slices, transposes
(layout prep), and sums the 8 partial [64,2] outputs.
"""

import sys

import numpy as np

for _p in ("/opt/trn_rl_repo",):
    if _p not in sys.path:
        sys.path.insert(0, _p)

# If the environment sets BASS_TRACE but lacks antenv.axon_hooks (this agent
# image does), run_bass_kernel_spmd would crash on import; pre-seed a no-op
# module so tracing degrades gracefully instead.
try:  # pragma: no cover
    import antenv.axon_hooks  # noqa: F401
except Exception:  # pragma: no cover
    import types as _types

    _m = _types.ModuleType("antenv.axon_hooks")
    _m._hook = None
    _m.set_axon_ntff_profile_hook = lambda h: setattr(_m, "_hook", h)
    _m.get_axon_ntff_profile_hook = lambda: _m._hook
    sys.modules["antenv.axon_hooks"] = _m

B, S, H = 64, 128, 1024
E, L = 8, 2
NCORES = 8
OC = H // NCORES          # dense-output slice per core
KT = H // 128             # contraction tiles
P = 128

# Stage-1 matmul layout:
#   "A": psum [B, (e,oc)]  - lhsT = xT k-tile  [128,  64], 16 MMs of N=512
#   "B": psum [(oc), (e,b)] - lhsT = w1 block  [128, 128], 64 MMs of N=64
LAYOUT = "A"

_cached = None


def _build(layout=LAYOUT):
    from contextlib import ExitStack

    import concourse.bass as bass
    import concourse.tile as tile
    from concourse import bacc, mybir

    F32 = mybir.dt.float32
    AF = mybir.ActivationFunctionType
    OP = mybir.AluOpType

    nc = bacc.Bacc("TRN2", target_bir_lowering=False, debug=False,
                   num_devices=NCORES)

    xt_d = nc.dram_tensor("xt", [P, KT, B], F32, kind="ExternalInput")
    w1_d = nc.dram_tensor("w1", [P, KT, E, OC], F32, kind="ExternalInput")
    gt_d = nc.dram_tensor("gt", [E, B], F32, kind="ExternalInput")
    gc_d = nc.dram_tensor("gc", [B, E], F32, kind="ExternalInput")
    db_d = nc.dram_tensor("db", [E, OC], F32, kind="ExternalInput")
    ow_d = nc.dram_tensor("ow", [E, L * OC], F32, kind="ExternalInput")
    ob_d = nc.dram_tensor("ob", [E, L], F32, kind="ExternalInput")
    out_d = nc.dram_tensor("out", [B, L], F32, kind="ExternalOutput")

    with tile.TileContext(nc) as tc, ExitStack() as ctx:
        consts = ctx.enter_context(tc.tile_pool(name="consts", bufs=1))
        wpool = ctx.enter_context(tc.tile_pool(name="wpool", bufs=1))
        mixp = ctx.enter_context(tc.tile_pool(name="mixp", bufs=2))
        smallp = ctx.enter_context(tc.tile_pool(name="smallp", bufs=1))
        psy = ctx.enter_context(tc.tile_pool(name="psy", bufs=1, space="PSUM"))
        pss = ctx.enter_context(tc.tile_pool(name="pss", bufs=1, space="PSUM"))

        # xt gates every stage-1 matmul: first on the sync HWDGE ring.
        xt_t = consts.tile([P, KT, B], F32)
        nc.sync.dma_start(out=xt_t, in_=xt_d.ap())
        # big weight slice, chunked by k so matmuls can start early
        w1_t = wpool.tile([P, KT, E, OC], F32)
        for k in range(KT):
            nc.sync.dma_start(out=w1_t[:, k], in_=w1_d.ap()[:, k])

        # small constants on the scalar HWDGE ring (no SWDGE init, and they
        # don't queue behind the 4 MiB w1 stream)
        gt_t = consts.tile([E, B], F32)
        nc.scalar.dma_start(out=gt_t, in_=gt_d.ap())
        gc_t = consts.tile([B, E], F32)
        nc.scalar.dma_start(out=gc_t, in_=gc_d.ap())
        db_t = consts.tile([E, OC], F32)
        nc.scalar.dma_start(out=db_t, in_=db_d.ap())
        ow_t = consts.tile([E, L * OC], F32)
        nc.scalar.dma_start(out=ow_t, in_=ow_d.ap())
        ob_t = consts.tile([E, L], F32)
        nc.scalar.dma_start(out=ob_t, in_=ob_d.ap())

        # ---- small matmuls first so their consumers unblock early ----
        psum_db = pss.tile([B, OC], F32)
        nc.tensor.matmul(psum_db[:], gt_t[:], db_t[:], start=True, stop=True)
        psum_ow = pss.tile([B, L, OC], F32)
        nc.tensor.matmul(
            psum_ow[:].rearrange("b l o -> b (l o)"),
            gt_t[:], ow_t[:], start=True, stop=True,
        )
        psum_ob = pss.tile([B, L], F32)
        nc.tensor.matmul(psum_ob[:], gt_t[:], ob_t[:], start=True, stop=True)
        sdb_t = smallp.tile([B, OC], F32)
        nc.scalar.copy(sdb_t[:], psum_db[:])
        sob_t = smallp.tile([B, L], F32)
        nc.scalar.copy(sob_t[:], psum_ob[:])

        # ---- stage 1: y_e[b, oc] = x . dense_w[e, oc_slice, :] ----
        # h-outer: the first half's accumulation (experts 0-3) finishes after
        # 8 matmuls, so its gate-mix overlaps the second half's matmuls.
        psum_y = psy.tile([B, E, OC], F32)              # [64, 8, 128]
        acc = sdb_t
        for h in range(2):
            for k in range(KT):
                nc.tensor.matmul(
                    psum_y[:, h * 4:(h + 1) * 4, :],
                    xt_t[:, k, :],
                    w1_t[:, k, h * 4:(h + 1) * 4, :],
                    start=(k == 0),
                    stop=(k == KT - 1),
                )
            # gate mix for this half: acc += sum_e g[:,e] * y_e
            for e in range(4 * h, 4 * h + 4):
                nxt = mixp.tile([B, OC], F32, tag="acc")
                nc.vector.scalar_tensor_tensor(
                    out=nxt,
                    in0=psum_y[:, e, :],
                    scalar=gc_t[:, e:e + 1],
                    in1=acc[:],
                    op0=OP.mult,
                    op1=OP.add,
                )
                acc = nxt

        t_t = smallp.tile([B, OC], F32)
        nc.scalar.activation(t_t[:], acc[:], AF.Tanh)

        # ---- stage 2 ----
        # partial[b,l] = sum_oc t[b,oc] * sel_ow[b,l,oc]  (+ sel_ob)
        # NOTE: InstTensorTensorReduce faults TRN2 here; scalar_tensor_tensor
        # with accum_out (free-dim sum) is the reliable path.
        out_t = smallp.tile([B, L], F32)
        pre_t = smallp.tile([B, L], F32)
        dump = smallp.tile([B, OC], F32)
        for l in range(L):
            nc.vector.scalar_tensor_tensor(
                out=dump[:],
                in0=psum_ow[:, l, :],
                scalar=1.0,
                in1=t_t[:],
                op0=OP.mult,
                op1=OP.mult,
                accum_out=pre_t[:, l:l + 1],
            )
        nc.vector.tensor_add(out_t[:], pre_t[:], sob_t[:])

        nc.sync.dma_start(out=out_d.ap(), in_=out_t[:])

    nc.compile()
    return nc


def _prep_inputs(X, gates, dense_w, dense_b, out_w, out_b):
    """Host-side layout prep (slice/transpose only) -> per-core input maps."""
    X = np.asarray(X, dtype=np.float32)
    gates = np.asarray(gates, dtype=np.float32)
    dense_w = np.asarray(dense_w, dtype=np.float32)
    dense_b = np.asarray(dense_b, dtype=np.float32)
    out_w = np.asarray(out_w, dtype=np.float32)
    out_b = np.asarray(out_b, dtype=np.float32)

    xcls = X[:, 0, :]                                     # [B, H]
    # xt[i_lo, k, b] = x[b, k*128 + i_lo]
    xt = np.ascontiguousarray(xcls.T.reshape(KT, P, B).transpose(1, 0, 2))
    gt = np.ascontiguousarray(gates.T)                    # [E, B]
    gc = np.ascontiguousarray(gates)                      # [B, E]

    in_maps = []
    for c in range(NCORES):
        sl = slice(c * OC, (c + 1) * OC)
        # w1[i_lo, k, e, oc] = dense_w[e, c*OC + oc, k*128 + i_lo]
        w1 = np.ascontiguousarray(
            dense_w[:, sl, :].reshape(E, OC, KT, P).transpose(3, 2, 0, 1)
        )
        ow = np.ascontiguousarray(out_w[:, :, sl].reshape(E, L * OC))
        ob = out_b if c == 0 else np.zeros_like(out_b)
        in_maps.append({
            "xt": xt,
            "w1": w1,
            "gt": gt,
            "gc": gc,
            "db": np.ascontiguousarray(dense_b[:, sl]),
            "ow": ow,
            "ob": np.ascontiguousarray(ob),
        })
    return in_maps


def _run(in_maps, trace=False, tmpdir=None):
    global _cached
    from concourse.bass_utils import run_bass_kernel_spmd

    if _cached is None:
        _cached = _build()
    res = run_bass_kernel_spmd(
        _cached, in_maps, list(range(NCORES)), trace=trace, tmpdir=tmpdir,
    )
    return res


def kernel(X, gates, dense_w, dense_b, out_w, out_b):
    in_maps = _prep_inputs(X, gates, dense_w, dense_b, out_w, out_b)
    res = _run(in_maps)
    acc = np.zeros((B, L), dtype=np.float64)
    for c in range(NCORES):
        acc += res.results[c]["out"].astype(np.float64)
    return acc.astype(np.float32)
